# revision 1
# baseline (speedup 1.0000x reference)
"""Multi-class 3D DICE loss on 8 Trainium2 NeuronCores.

Data-parallel over the subject (batch) axis: core b reduces subject b's
[C=4, 64, 128, 128] volumes to per-class (inter, mask_sum, out_sum) partial
sums; the host applies the ~10-flop DICE scalar tail and averages the 8
per-subject losses.

Per-core layout: each input tensor is viewed as [128, 32768] where
partition q = c*32 + p (class c in partition block [32c, 32c+32)).
Per chunk (descending sizes, 4 MiB DMAs in steady state):
  - DVE  scalar_tensor_tensor: partial sums of output*masks   (inter)
  - DVE  tensor_reduce: partial sums of output
  - ACT  activation(Copy, accum_out): partial sums of masks
  - PE   collapses partition blocks into per-class sums with one matmul
"""

import os
import sys
from contextlib import ExitStack

import numpy as np

for _p in ("/opt/trn_rl_repo",):
    if _p not in sys.path and os.path.isdir(_p):
        sys.path.insert(0, _p)

import concourse.bass as bass  # noqa: E402
import concourse.tile as tile  # noqa: E402
from concourse import bacc, mybir  # noqa: E402
from concourse.bass_utils import run_bass_kernel_spmd  # noqa: E402

N_CORES = 8
B, C = 8, 4
SPATIAL = 64 * 128 * 128            # 1,048,576 per (subject, class)
P = 128                             # SBUF partitions = C * 32
COLS = (C * SPATIAL) // P           # 32768 elements per partition
# Descending chunk schedule: big DMAs (4 MiB) for bandwidth in the steady
# state, small chunks at the end so the post-last-byte compute tail is tiny.
CHUNKS = [8192, 8192, 8192, 4096, 2048, 1024, 512, 512]
BIG_FD = 4096  # chunks >= this land in the big pools, the rest in tail pools
assert sum(CHUNKS) == COLS
NCHUNK = len(CHUNKS)
EPS = 1e-7
F32 = mybir.dt.float32


def _dice_body(ctx: ExitStack, tc: "tile.TileContext", out_ap, x_ap, m_ap):
    nc = tc.nc
    add = mybir.AluOpType.add
    mult = mybir.AluOpType.mult
    Copy = mybir.ActivationFunctionType.Copy

    consts = ctx.enter_context(tc.tile_pool(name="consts", bufs=1))
    xpool = ctx.enter_context(tc.tile_pool(name="xin", bufs=2))
    mpool = ctx.enter_context(tc.tile_pool(name="min", bufs=2))
    xtail = ctx.enter_context(tc.tile_pool(name="xtail", bufs=3))
    mtail = ctx.enter_context(tc.tile_pool(name="mtail", bufs=3))
    small = ctx.enter_context(tc.tile_pool(name="small", bufs=1))
    psum = ctx.enter_context(tc.tile_pool(name="psum", bufs=1, space="PSUM"))

    # Block indicator: ind[q, c] = 1.0 iff q // 32 == c. lhsT for the
    # partition-block -> per-class collapse.
    ind = consts.tile([P, C], F32)
    nc.vector.memset(ind[:], 0.0)
    for c in range(C):
        nc.vector.memset(ind[c * 32 : (c + 1) * 32, c : c + 1], 1.0)

    # Per-chunk partial sums (column j <- chunk j); no cross-chunk deps.
    # One fused accumulator: cols [0,N) = sum(x*m), [N,2N) = sum(m),
    # [2N,3N) = sum(x) — lets a single matmul collapse all three.
    acc = small.tile([P, 3 * NCHUNK], F32)
    # Engines must write their full elementwise result somewhere; stride-0
    # broadcast dummies avoid real [P, fd] scratch tiles (HW-verified).
    dve_dummy = small.tile([P, 1], F32)
    act_dummy = small.tile([P, 1], F32)

    off = 0
    for j, fd in enumerate(CHUNKS):
        big = fd >= BIG_FD
        xt = (xpool if big else xtail).tile([P, fd], F32, tag="xt")
        nc.sync.dma_start(out=xt[:], in_=x_ap[:, off : off + fd])
        mt = (mpool if big else mtail).tile([P, fd], F32, tag="mt")
        nc.sync.dma_start(out=mt[:], in_=m_ap[:, off : off + fd])
        off += fd

        # inter partials on DVE: out = (x*1)*m, accum = X-reduce(out).
        nc.vector.scalar_tensor_tensor(
            out=dve_dummy.broadcast_to((P, fd)),
            in0=xt[:],
            scalar=1.0,
            in1=mt[:],
            op0=mult,
            op1=mult,
            accum_out=acc[:, j : j + 1],
        )
        nc.scalar.activation(
            out=act_dummy.broadcast_to((P, fd)),
            in_=mt[:],
            func=Copy,
            accum_out=acc[:, NCHUNK + j : NCHUNK + j + 1],
        )
        # x-sum on DVE. Keep each 32 B accumulator word single-engine: cols
        # 0-7 DVE, 8-15 ACT, 16-23 DVE — mixing engines within one word
        # produced intermittent lost-update corruption on HW.
        nc.vector.tensor_reduce(
            acc[:, 2 * NCHUNK + j : 2 * NCHUNK + j + 1],
            xt[:],
            axis=mybir.AxisListType.X,
            op=add,
        )

    # Partition blocks -> per-(class, quantity, chunk) sums in one matmul,
    # then one PSUM-side reduce over the chunk axis -> [4, 3] class sums
    # (inter, msum, xsum). The remaining ~10-flop scalar tail runs on the
    # host during unshard.
    ps = psum.tile([C, 3 * NCHUNK], F32)
    nc.tensor.matmul(out=ps[:], lhsT=ind[:], rhs=acc[:], start=True, stop=True)
    sums = small.tile([C, 3], F32)
    nc.vector.tensor_reduce(
        sums[:],
        ps[:].rearrange("c (q n) -> c q n", q=3),
        axis=mybir.AxisListType.X,
        op=add,
    )
    nc.sync.dma_start(out=out_ap, in_=sums[:])


_CACHE: dict[str, object] = {}


def _build():
    if "nc" in _CACHE:
        return _CACHE["nc"]
    nc = bacc.Bacc("TRN2", target_bir_lowering=False, debug=False)
    x = nc.dram_tensor("x", [P, COLS], F32, kind="ExternalInput").ap()
    m = nc.dram_tensor("m", [P, COLS], F32, kind="ExternalInput").ap()
    out = nc.dram_tensor("class_sums", [C, 3], F32, kind="ExternalOutput").ap()
    with tile.TileContext(nc) as tc:
        with ExitStack() as ctx:
            _dice_body(ctx, tc, out, x, m)
    nc.compile()
    _CACHE["nc"] = nc
    return nc


def _in_maps(output: np.ndarray, masks: np.ndarray):
    output = np.ascontiguousarray(output, dtype=np.float32)
    masks = np.ascontiguousarray(masks, dtype=np.float32)
    return [
        {"x": output[b].reshape(P, COLS), "m": masks[b].reshape(P, COLS)}
        for b in range(N_CORES)
    ]


def _finish(cs: np.ndarray) -> np.float32:
    """Per-subject scalar tail (fp32, mirrors the reference ordering).

    cs: [C, 3] device output — columns (inter, mask_sum, x_sum) per class.
    """
    cs = cs.astype(np.float32)
    inter, msum, xsum = cs[:, 0], cs[:, 1], cs[:, 2]
    w = np.float32(1.0) / (msum * msum + np.float32(EPS))
    total = xsum + msum
    nom = (w * inter).sum(dtype=np.float32)
    den = (w * total + np.float32(EPS)).sum(dtype=np.float32)
    return np.float32(1.0) - np.float32(2.0) * nom / den


def run_sharded(output: np.ndarray, masks: np.ndarray, **spmd_kwargs):
    """Run the SPMD kernel; returns (loss[1], BassKernelResults)."""
    nc = _build()
    res = run_bass_kernel_spmd(
        nc, _in_maps(output, masks), list(range(N_CORES)), **spmd_kwargs
    )
    per_subj = np.array(
        [_finish(res.results[b]["class_sums"]) for b in range(N_CORES)],
        dtype=np.float32,
    )
    loss = (per_subj.sum(dtype=np.float32) / np.float32(B)).reshape(1)
    return loss.astype(np.float32), res


def kernel(output: np.ndarray, masks: np.ndarray) -> np.ndarray:
    loss, _ = run_sharded(output, masks)
    return loss



# revision 4
# speedup vs baseline: 1.1933x; 1.1933x over previous
"""Multi-class 3D DICE loss on 8 Trainium2 NeuronCores.

Data-parallel over the subject (batch) axis: core b reduces subject b's
[C=4, 64, 128, 128] volumes to per-class (inter, mask_sum, out_sum) partial
sums; the host applies the ~10-flop DICE scalar tail and averages the 8
per-subject losses.

Per-core layout: each input tensor is viewed as [128, 32768] where
partition q = c*32 + p (class c in partition block [32c, 32c+32)).
Per chunk (descending sizes, 4 MiB DMAs in steady state):
  - DVE  scalar_tensor_tensor: partial sums of output*masks   (inter)
  - ACT  activation(Copy, accum_out): partial sums of output
  - ACT  activation(Copy, accum_out): partial sums of masks
  - PE   collapses partition blocks into per-class sums with one matmul
Engine budget per 8 MiB chunk-pair (~19.5 us of DMA at ~430 GB/s):
DVE one pass ~8.7 us, ACT two passes ~14.2 us — both stay ahead of the
DMA stream, so the stream never stalls on buffer reuse.
"""

import os
import sys
from contextlib import ExitStack

import numpy as np

for _p in ("/opt/trn_rl_repo",):
    if _p not in sys.path and os.path.isdir(_p):
        sys.path.insert(0, _p)

import concourse.bass as bass  # noqa: E402
import concourse.tile as tile  # noqa: E402
from concourse import bacc, mybir  # noqa: E402
from concourse.bass_utils import run_bass_kernel_spmd  # noqa: E402

N_CORES = 8
B, C = 8, 4
SPATIAL = 64 * 128 * 128            # 1,048,576 per (subject, class)
P = 128                             # SBUF partitions = C * 32
COLS = (C * SPATIAL) // P           # 32768 elements per partition
# Descending chunk schedule: big DMAs (4 MiB) for bandwidth in the steady
# state, small chunks at the end so the post-last-byte compute tail is tiny.
CHUNKS = [8192, 8192, 8192, 4096, 2048, 1024, 512, 512]
BIG_FD = 4096  # chunks >= this land in the big pools, the rest in tail pools
assert sum(CHUNKS) == COLS
NCHUNK = len(CHUNKS)
EPS = 1e-7
F32 = mybir.dt.float32


def _dice_body(ctx: ExitStack, tc: "tile.TileContext", out_ap, x_ap, m_ap):
    nc = tc.nc
    add = mybir.AluOpType.add
    mult = mybir.AluOpType.mult
    Copy = mybir.ActivationFunctionType.Copy

    consts = ctx.enter_context(tc.tile_pool(name="consts", bufs=1))
    xpool = ctx.enter_context(tc.tile_pool(name="xin", bufs=2))
    mpool = ctx.enter_context(tc.tile_pool(name="min", bufs=2))
    xtail = ctx.enter_context(tc.tile_pool(name="xtail", bufs=3))
    mtail = ctx.enter_context(tc.tile_pool(name="mtail", bufs=3))
    small = ctx.enter_context(tc.tile_pool(name="small", bufs=1))
    psum = ctx.enter_context(tc.tile_pool(name="psum", bufs=1, space="PSUM"))

    # Block indicator: ind[q, c] = 1.0 iff q // 32 == c. lhsT for the
    # partition-block -> per-class collapse.
    ind = consts.tile([P, C], F32)
    nc.vector.memset(ind[:], 0.0)
    for c in range(C):
        nc.vector.memset(ind[c * 32 : (c + 1) * 32, c : c + 1], 1.0)

    # Per-chunk partial sums (column j <- chunk j); no cross-chunk deps.
    # One fused accumulator: cols [0,N) = sum(x*m), [N,2N) = sum(m),
    # [2N,3N) = sum(x) — lets a single matmul collapse all three.
    acc = small.tile([P, 3 * NCHUNK], F32)
    # Engines must write their full elementwise result somewhere; stride-0
    # broadcast dummies avoid real [P, fd] scratch tiles (HW-verified).
    dve_dummy = small.tile([P, 1], F32)
    act_dummy = small.tile([P, 1], F32)
    act_dummy2 = small.tile([P, 1], F32)

    off = 0
    for j, fd in enumerate(CHUNKS):
        big = fd >= BIG_FD
        xt = (xpool if big else xtail).tile([P, fd], F32, tag="xt")
        nc.sync.dma_start(out=xt[:], in_=x_ap[:, off : off + fd])
        mt = (mpool if big else mtail).tile([P, fd], F32, tag="mt")
        nc.sync.dma_start(out=mt[:], in_=m_ap[:, off : off + fd])
        off += fd

        # inter partials on DVE: out = (x*1)*m, accum = X-reduce(out).
        nc.vector.scalar_tensor_tensor(
            out=dve_dummy.broadcast_to((P, fd)),
            in0=xt[:],
            scalar=1.0,
            in1=mt[:],
            op0=mult,
            op1=mult,
            accum_out=acc[:, j : j + 1],
        )
        # Both plain sums on ACT (x first — its DMA lands before m's).
        # Keep each 32 B accumulator word single-engine: cols 0-7 DVE,
        # 8-23 ACT — mixing engines within one word produced intermittent
        # lost-update corruption on HW.
        nc.scalar.activation(
            out=act_dummy2.broadcast_to((P, fd)),
            in_=xt[:],
            func=Copy,
            accum_out=acc[:, 2 * NCHUNK + j : 2 * NCHUNK + j + 1],
        )
        nc.scalar.activation(
            out=act_dummy.broadcast_to((P, fd)),
            in_=mt[:],
            func=Copy,
            accum_out=acc[:, NCHUNK + j : NCHUNK + j + 1],
        )

    # Partition blocks -> per-(class, quantity, chunk) sums in one matmul,
    # then one PSUM-side reduce over the chunk axis -> [4, 3] class sums
    # (inter, msum, xsum). The remaining ~10-flop scalar tail runs on the
    # host during unshard.
    ps = psum.tile([C, 3 * NCHUNK], F32)
    nc.tensor.matmul(out=ps[:], lhsT=ind[:], rhs=acc[:], start=True, stop=True)
    sums = small.tile([C, 3], F32)
    nc.vector.tensor_reduce(
        sums[:],
        ps[:].rearrange("c (q n) -> c q n", q=3),
        axis=mybir.AxisListType.X,
        op=add,
    )
    nc.sync.dma_start(out=out_ap, in_=sums[:])


_CACHE: dict[str, object] = {}


def _build():
    if "nc" in _CACHE:
        return _CACHE["nc"]
    nc = bacc.Bacc("TRN2", target_bir_lowering=False, debug=False)
    x = nc.dram_tensor("x", [P, COLS], F32, kind="ExternalInput").ap()
    m = nc.dram_tensor("m", [P, COLS], F32, kind="ExternalInput").ap()
    out = nc.dram_tensor("class_sums", [C, 3], F32, kind="ExternalOutput").ap()
    with tile.TileContext(nc) as tc:
        with ExitStack() as ctx:
            _dice_body(ctx, tc, out, x, m)
    nc.compile()
    _CACHE["nc"] = nc
    return nc


def _in_maps(output: np.ndarray, masks: np.ndarray):
    output = np.ascontiguousarray(output, dtype=np.float32)
    masks = np.ascontiguousarray(masks, dtype=np.float32)
    return [
        {"x": output[b].reshape(P, COLS), "m": masks[b].reshape(P, COLS)}
        for b in range(N_CORES)
    ]


def _finish(cs: np.ndarray) -> np.float32:
    """Per-subject scalar tail (fp32, mirrors the reference ordering).

    cs: [C, 3] device output — columns (inter, mask_sum, x_sum) per class.
    """
    cs = cs.astype(np.float32)
    inter, msum, xsum = cs[:, 0], cs[:, 1], cs[:, 2]
    w = np.float32(1.0) / (msum * msum + np.float32(EPS))
    total = xsum + msum
    nom = (w * inter).sum(dtype=np.float32)
    den = (w * total + np.float32(EPS)).sum(dtype=np.float32)
    return np.float32(1.0) - np.float32(2.0) * nom / den


def run_sharded(output: np.ndarray, masks: np.ndarray, **spmd_kwargs):
    """Run the SPMD kernel; returns (loss[1], BassKernelResults)."""
    nc = _build()
    res = run_bass_kernel_spmd(
        nc, _in_maps(output, masks), list(range(N_CORES)), **spmd_kwargs
    )
    per_subj = np.array(
        [_finish(res.results[b]["class_sums"]) for b in range(N_CORES)],
        dtype=np.float32,
    )
    loss = (per_subj.sum(dtype=np.float32) / np.float32(B)).reshape(1)
    return loss.astype(np.float32), res


def kernel(output: np.ndarray, masks: np.ndarray) -> np.ndarray:
    loss, _ = run_sharded(output, masks)
    return loss

